# revision 39
# baseline (speedup 1.0000x reference)
"""GAT (graph attention) forward on 8 TRN2 NeuronCores, Bass/Tile.

Sharding: target nodes (rows of the output) split into 8 blocks of 512.
Each core redundantly computes the projected features h for ALL nodes
(cheap: one K=128 matmul chain) and then its own 512-row slice of the
attention + aggregation + skip + ELU.  No collectives.

Score factorization trick: with z[m,n] = s_tgt[m] + s_src[n] and
leaky(z) = max(z, 0.2 z),

    exp(leaky(z)) = e^{0.2 s_src[n]} * u_m * max(w_n, q_m)

where u = e^{s_tgt}, q = e^{-0.8 s_tgt}, w = e^{0.8 s_src}.  The leading
per-target factor cancels in the softmax normalization, so per (head,
chunk) the kernel runs one 2-op DVE tensor_scalar
(t2 = (w max q) * u, 2x perf mode) and one mask tensor_tensor
(et = t2 * M01, 2x mode, slab-batched).  This is the DVE wall: cayman
has no 2x uop for the fused scalar_tensor_tensor (measured 1x), GpSimd
tensor ops contend ~12x with concurrent DVE work, and the Act engine
cannot multiply two tensors - so ~1 masked-scored element per
lane-cycle is the hard elementwise floor and the DVE paces the kernel.

The aggregation matmul streams et against the stationary h_ext (h with
a ones column) so the softmax denominator falls out of the same PSUM
accumulation; a per-head transpose + batched reciprocal + scaled copy
assembles the normalized output columns.

Head 0's elementwise + aggregation is fused into the phase-1
projection loop (only 3 head periods are exposed), the skip+bias
matmuls are hoisted off the tail into PSUM during head 1, and the ELU
runs as relu/exp legs on the Act engine with only two DVE ops.

Everything matmul runs in bf16 (scores included; rel-err ~3.4e-3
against the 2e-2 budget), which halves the input DMA and enables FWL
weight loads.  The mask is transposed and converted to multiplicative
0/1 bf16 on the host so the device only ever does contiguous row DMA.
All bulk DMA rides the SP HWDGE queue - mask slabs interleaved with xT
pieces so each lands just before its first consumer; the Act queue is
NOT used for bulk data because its DMA_DIRECT2D dispatch occupies the
Act engine ~2.5us per slab.  The S3_LW (weight-load) instruction can
carry only one semaphore wait, so _split_multi_waits rewrites any
instruction Tile scheduled with 2+.
"""

import numpy as np
from contextlib import ExitStack

import concourse.bass as bass
import concourse.mybir as mybir
from concourse.tile import TileContext
from concourse.masks import make_identity
from concourse.bass_utils import run_bass_kernel_spmd

F32 = mybir.dt.float32
F32R = mybir.dt.float32r
BF16 = mybir.dt.bfloat16
AF = mybir.ActivationFunctionType
OP = mybir.AluOpType

N, FIN, H, FOUT = 4096, 128, 4, 64
G = H * FOUT
NCORES = 8
NLOC = N // NCORES          # local target rows per core
NCH = N // 128              # source (m) chunks
LCH = NLOC // 128           # local output row chunks
HE = FOUT + 1               # hu_ext columns (u column at index FOUT)
XPC = 8                     # phase-1 chunks per xT DMA piece
NXP = NCH // XPC            # xT DMA pieces

# GpSimd tensor ops contend catastrophically with concurrent DVE ops
# (measured: DVE TS slows 12x while a gp TT runs), so ALL elementwise
# work stays on the DVE.
GP_BLK = []
DV_BLK = [(0, 4), (4, 4), (8, 16), (24, 8)]  # DVE TT slabs
ALL_BLK = DV_BLK
START_C = 0
STOP_C = NCH - 1


def build_program():
    # cpack: xT | xTloc | wsrc | wproj|wtgt | wskip   (bf16)
    cw = N + NLOC + H + G + H + G

    nc = bass.Bass()
    d_cpack = nc.declare_dram_parameter("cpack", [128, cw], BF16, isOutput=False)
    d_mask = nc.declare_dram_parameter("mask01", [N, NLOC], BF16, isOutput=False)
    d_sel = nc.declare_dram_parameter("selc", [4, 4 * 128], BF16, isOutput=False)
    d_bias = nc.declare_dram_parameter("biasr", [1, G], F32R, isOutput=False)
    d_out = nc.declare_dram_parameter("out", [NLOC, G], F32, isOutput=True)

    with TileContext(nc) as tc, ExitStack() as ctx:
        cp = ctx.enter_context(tc.tile_pool(name="const", bufs=1))
        sb_cpack = cp.tile([128, cw], BF16, tag="cpack")
        o = 0
        xTr = sb_cpack[:, o:o + N]; o += N
        xTlocr = sb_cpack[:, o:o + NLOC]; o += NLOC
        wsrcr = sb_cpack[:, o:o + H]; o += H
        wpsr = sb_cpack[:, o:o + G + H]; o += G + H      # wproj | wtgt merged
        wskipr = sb_cpack[:, o:o + G]; o += G
        browr = cp.tile([1, G], F32R, tag="brow")

        sb_sel = cp.tile([4, 4 * 128], BF16, tag="sel")   # one-hot head rows
        sb_id = cp.tile([128, 128], F32, tag="ident")
        sb_hp = cp.tile([128, NCH * H * HE], BF16, tag="hext")   # h | 1
        sb_w = cp.tile([128, H * NLOC], BF16, tag="wbc")      # e^{0.8 s_src}
        sb_wrow = cp.tile([4, NLOC], BF16, tag="wrow")
        sb_ones1 = cp.tile([1, 128], F32R, tag="ones1")
        sb_q = cp.tile([128, NCH * H], F32, tag="qexp")       # e^{-0.8 s_tgt}
        sb_u = cp.tile([128, NCH * H], F32, tag="uexp")       # e^{s_tgt}
        sb_m = [cp.tile([128, nb * NLOC], BF16, tag=f"m{k}", name=f"m{k}")
                for k, (j0, nb) in enumerate(ALL_BLK)]

        # h_ext view [128, c, h, HE]
        hp4 = sb_hp[:].rearrange("p (c h w) -> p c h w", h=H, w=HE)

        # ---- DMA: everything big rides the SP HWDGE queue, xT pieces
        # interleaved with mask slabs (each xT piece lands well before the
        # phase-1 chunk that needs it; the Act queue is NOT used for bulk
        # data because its DMA_DIRECT2D dispatch occupies the Act engine
        # for ~2.5us per slab, starving phase-1 exps/copies).
        def dma_mask(k):
            j0, nb = ALL_BLK[k]
            mv = sb_m[k][:].rearrange("p (c n) -> p c n", n=NLOC)
            dv = d_mask[j0 * 128:(j0 + nb) * 128, :].rearrange(
                "(c p) n -> p c n", p=128)
            nc.sync.dma_start(out=mv, in_=dv)

        nc.sync.dma_start(out=sb_cpack[:, N:N + NLOC + H],
                          in_=d_cpack[:, N:N + NLOC + H])
        nc.sync.dma_start(out=sb_cpack[:, N + NLOC + H:cw],
                          in_=d_cpack[:, N + NLOC + H:cw])
        nc.scalar.dma_start(out=sb_sel[:], in_=d_sel[:])
        nc.scalar.dma_start(out=browr[:], in_=d_bias[:])
        mj = 0
        for p in range(NXP):
            w0 = p * XPC * 128
            nc.sync.dma_start(out=sb_cpack[:, w0:w0 + XPC * 128],
                              in_=d_cpack[:, w0:w0 + XPC * 128])
            while mj < len(ALL_BLK) and mj <= 2 * p:
                dma_mask(mj); mj += 1
        while mj < len(ALL_BLK):
            dma_mask(mj); mj += 1

        make_identity(nc, sb_id[:])
        nc.vector.memset(sb_ones1[:].bitcast(F32), 1.0)
        # ones column of h_ext: the scaled hu copy then yields u*1 = u in
        # the denominator column for free
        nc.vector.memset(hp4[:, :, :, FOUT:FOUT + 1], 1.0)

        def q_ap(c, hh):
            return sb_q[:, c * H + hh:c * H + hh + 1]

        def hu_lhsT(c, hh):
            return hp4[:, c:c + 1, hh:hh + 1, 0:HE]

        # ---- phase 0: b = s_src(local), w = e^{0.8 b} broadcast -----------
        # (pso opened first so po banks coexist with phase-1 ph banks)
        pso = ctx.enter_context(tc.tile_pool(name="pso", bufs=1, space="PSUM"))
        po = [pso.tile([HE, NLOC], F32, tag=f"po{hh}", name=f"po{hh}")
              for hh in range(H)]

        with tc.tile_pool(name="ps0", bufs=1, space="PSUM") as ps0:
            pb = ps0.tile([4, NLOC], F32, tag="pb")
            nc.tensor.matmul(pb[:], wsrcr, xTlocr, start=True, stop=True)
            nc.scalar.activation(sb_wrow[:], pb[:], AF.Exp, scale=0.8)
            for hh in range(H):
                pwb = ps0.tile([128, NLOC], F32, tag=f"pwb{hh % 2}",
                               name=f"pwb{hh}")
                nc.tensor.matmul(pwb[:], sb_sel[0:4, hh * 128:(hh + 1) * 128],
                                 sb_wrow[0:4, :], start=True, stop=True)
                nc.scalar.copy(sb_w[:, hh * NLOC:(hh + 1) * NLOC], pwb[:])

        # et slabs for the attention loop: written per-chunk by the DVE
        # scalar_tensor_tensor, consumed per-chunk by the PE.  bufs=2 so the
        # next head's DVE work overlaps this head's PE consumption.
        wpt = ctx.enter_context(tc.tile_pool(name="workt2", bufs=1))
        wp = ctx.enter_context(tc.tile_pool(name="work", bufs=2))
        fp = ctx.enter_context(tc.tile_pool(name="fin", bufs=1))
        fp2 = ctx.enter_context(tc.tile_pool(name="fin2", bufs=2))
        afs = [fp.tile([128, G], F32, tag=f"af{li}", name=f"af{li}")
               for li in range(LCH)]

        def blk_of(c):
            return next(k for k, (j0, nb) in enumerate(ALL_BLK)
                        if j0 <= c < j0 + nb)

        t2s = {}             # (hh, k) -> t2 slab (all blocks)
        ets = {}             # (hh, k) -> et slab

        def emit_ts(hh, c):
            # t2 = max(w, q) on DVE (2x mode), written into the block slab
            k = blk_of(c)
            j0, nb = ALL_BLK[k]
            if (hh, k) not in t2s:
                t2s[(hh, k)] = wpt.tile([128, nb * NLOC], BF16,
                                        tag=f"t2b{k}", name=f"t2_{hh}_{k}")
            t2 = t2s[(hh, k)]
            nc.vector.tensor_scalar(t2[:, (c - j0) * NLOC:(c - j0 + 1) * NLOC],
                                    sb_w[:, hh * NLOC:(hh + 1) * NLOC],
                                    q_ap(c, hh),
                                    sb_u[:, c * H + hh:c * H + hh + 1],
                                    OP.max, OP.mult)

        def emit_tt(hh, k):
            # et = t2 * M01 for a whole block: GpSimd for the GP blocks,
            # DVE for the rest
            j0, nb = ALL_BLK[k]
            et = wp.tile([128, nb * NLOC], BF16, tag=f"etb{k}",
                         name=f"et_{hh}_{k}")
            nc.vector.tensor_tensor(et[:], t2s[(hh, k)][:], sb_m[k][:], OP.mult)
            del t2s[(hh, k)]
            ets[(hh, k)] = et

        def emit_agg_blk(hh, k, start_c=START_C, stop_c=STOP_C):
            j0, nb = ALL_BLK[k]
            et = ets.pop((hh, k))
            for s in range(nb):
                c = j0 + s
                nc.tensor.matmul(po[hh][:], hu_lhsT(c, hh),
                                 et[:, s * NLOC:(s + 1) * NLOC],
                                 start=(c == start_c), stop=(c == stop_c))

        # ---- phase 1 fused with head 0 ------------------------------------
        # per chunk j: proj matmul -> u/q exps + h copy (Scalar) -> hu(head0)
        # + head-0 t2 (DVE); block TT fires at each block end (gpsimd TTs run
        # far ahead of their end-of-chain consumption).  DVE-block aggs trail
        # one chunk; gp-block aggs all run after the loop.
        with tc.tile_pool(name="ps1", bufs=4, space="PSUM") as ps1:
            for j in range(NCH):
                ph = ps1.tile([128, G + H], F32, tag="ph")
                nc.tensor.matmul(ph[:], xTr[:, j * 128:(j + 1) * 128], wpsr,
                                 start=True, stop=True)
                nc.scalar.activation(sb_u[:, j * H:(j + 1) * H],
                                     ph[:, G:G + H], AF.Exp)
                nc.scalar.activation(sb_q[:, j * H:(j + 1) * H],
                                     ph[:, G:G + H], AF.Exp, scale=-0.8)
                nc.scalar.copy(
                    hp4[:, j, :, 0:FOUT],
                    ph[:, 0:G].rearrange("p (h f) -> p h f", f=FOUT))
                emit_ts(0, j)
                for k, (j0, nb) in enumerate(ALL_BLK):
                    if j == j0 + nb - 1:
                        emit_tt(0, k)
                    if j == j0 + nb + 1:      # block k's aggs, trailing
                        emit_agg_blk(0, k)
            emit_agg_blk(0, len(ALL_BLK) - 1)

        # ---- heads 1..3 + per-head finalize -------------------------------
        pos_all = []
        with tc.tile_pool(name="psf", bufs=2, space="PSUM") as psf, \
             tc.tile_pool(name="psk", bufs=1, space="PSUM") as psk:
            # skip+bias for all li, hoisted off the tail critical path (the
            # PE runs these during head 1; results wait in PSUM)
            pskipb = psk.tile([128, LCH * G], F32, tag="pskipb")
            for li in range(LCH):
                nc.tensor.matmul(pskipb[:, li * G:(li + 1) * G],
                                 xTlocr[:, li * 128:(li + 1) * 128],
                                 wskipr, start=True, stop=False,
                                 skip_group_check=True)
                nc.tensor.matmul(pskipb[:, li * G:(li + 1) * G],
                                 sb_ones1[:], browr[0:1, :],
                                 start=False, stop=True, skip_group_check=True)

            def head_finalize(hh):
                # copy this head's accumulator out of PSUM so the PE can
                # transpose from SBUF, then per-li: transpose, 1/den, scaled
                # copy into af columns.  All overlapped with the next head.
                pos = cp.tile([HE, NLOC], F32, tag=f"pos{hh}", name=f"pos{hh}")
                for li in range(LCH):
                    nc.scalar.copy(pos[:, li * 128:(li + 1) * 128],
                                   po[hh][:, li * 128:(li + 1) * 128])
                pos_all.append(pos)
                ptb = psf.tile([128, LCH * HE], F32, tag="pt")
                for li in range(LCH):
                    nc.tensor.transpose(ptb[0:128, li * HE:li * HE + HE],
                                        pos[:, li * 128:(li + 1) * 128],
                                        sb_id[0:HE, 0:HE])
                rcpb = fp2.tile([128, LCH], F32, tag="rcp")
                nc.vector.reciprocal(
                    rcpb[:].rearrange("p (l o) -> p l o", o=1),
                    ptb[:].rearrange("p (l w) -> p l w", w=HE)[:, :, FOUT:FOUT + 1])
                for li in range(LCH):
                    nc.scalar.activation(afs[li][:, hh * FOUT:(hh + 1) * FOUT],
                                         ptb[:, li * HE:li * HE + FOUT],
                                         AF.Copy, scale=rcpb[:, li:li + 1])

            for hh in range(1, H):
                # DVE blocks: t2 + TT + aggs.  Head 3 rotates to end on the
                # small (4,4) block so the tail after its last DVE op is short.
                order = [2, 3, 0, 1] if hh == H - 1 else list(range(len(ALL_BLK)))
                st = ALL_BLK[order[0]][0]
                sp_j0, sp_nb = ALL_BLK[order[-1]]
                sp = sp_j0 + sp_nb - 1
                for k in order:
                    j0, nb = ALL_BLK[k]
                    for s in range(nb):
                        emit_ts(hh, j0 + s)
                    emit_tt(hh, k)
                    emit_agg_blk(hh, k, start_c=st, stop_c=sp)
                # previous head's finalize AFTER this head's PE chain: its
                # transposes sit behind these matmuls in the PE FIFO, so by
                # the time the PE reaches them the pos copy has long landed.
                head_finalize(hh - 1)
            head_finalize(H - 1)

            # ---- tail: ELU + store (per li, pipelined) --------------------
            for li in range(LCH):
                af = afs[li]
                nc.vector.tensor_tensor(af[:], af[:],
                                        pskipb[:, li * G:(li + 1) * G], OP.add)
                # ELU(z) = relu(z) + exp(-relu(-z)) - 1, relu/exp on Scalar
                rp = fp2.tile([128, G], F32, tag="rp")
                nc.scalar.activation(rp[:], af[:], AF.Relu)
                rn = fp2.tile([128, G], F32, tag="rn")
                nc.scalar.activation(rn[:], af[:], AF.Relu, scale=-1.0)
                ex = fp2.tile([128, G], F32, tag="ex")
                nc.scalar.activation(ex[:], rn[:], AF.Exp, scale=-1.0)
                nc.vector.tensor_tensor(af[:], rp[:], ex[:], OP.add)
                nc.vector.tensor_scalar(af[:], af[:], -1.0, None, OP.add)
                nc.sync.dma_start(out=d_out[li * 128:(li + 1) * 128, :], in_=af[:])

    _split_multi_waits(nc)
    return nc


def _split_multi_waits(nc):
    """walrus on this toolchain allows only one semaphore-wait command on
    most compute-engine instructions (S3_LW / S3D3_* structs).  Tile's
    scheduler freely emits 2+.  Move all but one wait onto an injected
    same-engine NoOp right before the offending instruction."""
    skip = (mybir.InstEventSemaphore,)
    k = 0
    for f in nc.m.functions:
        for blk in f.blocks:
            new = []
            for ins in blk.instructions:
                si = getattr(ins, "sync_info", None)
                w = list(si.on_wait) if si is not None and si.on_wait else []
                if len(w) > 1 and not isinstance(ins, skip):
                    for wx in w[:-1]:
                        nop = mybir.InstNoOp(name=f"waitsplit-{k}", ins=[], outs=[])
                        nop.engine = ins.engine
                        nop.sync_info = mybir.SyncInfo(on_wait=[wx], on_update=[])
                        new.append(nop)
                        k += 1
                    ins.sync_info = mybir.SyncInfo(on_wait=w[-1:],
                                                   on_update=list(si.on_update))
                new.append(ins)
            blk.instructions[:] = new


_PROG = None


def _get_prog():
    global _PROG
    if _PROG is None:
        _PROG = build_program()
    return _PROG


def make_in_maps(x, mask, proj_param, score_src, score_tgt, skip_w, bias):
    import ml_dtypes
    x = np.asarray(x, np.float32)
    mask = np.asarray(mask, np.float32)
    proj = np.asarray(proj_param, np.float32)
    a_src = np.asarray(score_src, np.float32)[:, :, 0]       # [H, FOUT]
    a_tgt = np.asarray(score_tgt, np.float32)[:, :, 0]
    skip = np.asarray(skip_w, np.float32)
    b = np.asarray(bias, np.float32)

    xT = np.ascontiguousarray(x.T)                           # [128, N]
    wproj = np.ascontiguousarray(proj.transpose(1, 0, 2).reshape(FIN, G))
    w_src = np.einsum('hif,hf->ih', proj, a_src)             # [FIN, H]
    w_tgt = np.einsum('hif,hf->ih', proj, a_tgt)
    wskip = np.ascontiguousarray(skip.T)                     # [128, G]
    mask01 = (mask == 0.0).astype(ml_dtypes.bfloat16)        # [N, N]

    sel = np.zeros((4, 4 * 128), ml_dtypes.bfloat16)
    for hh in range(H):
        sel[hh, hh * 128:(hh + 1) * 128] = 1

    in_maps = []
    for c in range(NCORES):
        r0 = c * NLOC
        cpack = np.concatenate(
            [xT, xT[:, r0:r0 + NLOC], w_src, wproj, w_tgt, wskip],
            axis=1).astype(ml_dtypes.bfloat16)
        in_maps.append({
            "cpack": np.ascontiguousarray(cpack),
            "mask01": np.ascontiguousarray(mask01[r0:r0 + NLOC, :].T),
            "selc": sel,
            "biasr": b.reshape(1, G).astype(np.float32),
        })
    return in_maps


def run(in_maps, trace=False, **kw):
    res = run_bass_kernel_spmd(_get_prog(), in_maps, list(range(NCORES)),
                               trace=trace, **kw)
    out = np.concatenate([res.results[c]["out"] for c in range(NCORES)], axis=0)
    return out, res


def kernel(x, mask, proj_param, score_src, score_tgt, skip_w, bias):
    in_maps = make_in_maps(x, mask, proj_param, score_src, score_tgt, skip_w, bias)
    out, _ = run(in_maps)
    return out.astype(np.float32)


# revision 40
# speedup vs baseline: 1.0209x; 1.0209x over previous
"""GAT (graph attention) forward on 8 TRN2 NeuronCores, Bass/Tile.

Sharding: target nodes (rows of the output) split into 8 blocks of 512.
Each core redundantly computes the projected features h for ALL nodes
(cheap: one K=128 matmul chain) and then its own 512-row slice of the
attention + aggregation + skip + ELU.  No collectives.

Score factorization trick: with z[m,n] = s_tgt[m] + s_src[n] and
leaky(z) = max(z, 0.2 z),

    exp(leaky(z)) = e^{0.2 s_src[n]} * u_m * max(w_n, q_m)

where u = e^{s_tgt}, q = e^{-0.8 s_tgt}, w = e^{0.8 s_src}.  The leading
per-target factor cancels in the softmax normalization, so per (head,
chunk) the kernel runs one 2-op DVE tensor_scalar
(t2 = (w max q) * u, 2x perf mode) and one mask tensor_tensor
(et = t2 * M01, 2x mode, slab-batched).  This is the DVE wall: cayman
has no 2x uop for the fused scalar_tensor_tensor (measured 1x), GpSimd
tensor ops contend ~12x with concurrent DVE work, and the Act engine
cannot multiply two tensors - so ~1 masked-scored element per
lane-cycle is the hard elementwise floor and the DVE paces the kernel.

The aggregation matmul streams et against the stationary h_ext (h with
a ones column) so the softmax denominator falls out of the same PSUM
accumulation; a per-head transpose + batched reciprocal + scaled copy
assembles the normalized output columns.

Head 0's elementwise + aggregation is fused into the phase-1
projection loop (only 3 head periods are exposed), the skip+bias
matmuls are hoisted off the tail into PSUM during head 1, and the ELU
runs as relu/exp legs on the Act engine with only two DVE ops.

Everything matmul runs in bf16 (scores included; rel-err ~3.4e-3
against the 2e-2 budget), which halves the input DMA and enables FWL
weight loads.  The mask is transposed and converted to multiplicative
0/1 bf16 on the host so the device only ever does contiguous row DMA.
All bulk DMA rides the SP HWDGE queue - mask slabs interleaved with xT
pieces so each lands just before its first consumer; the Act queue is
NOT used for bulk data because its DMA_DIRECT2D dispatch occupies the
Act engine ~2.5us per slab.  The S3_LW (weight-load) instruction can
carry only one semaphore wait, so _split_multi_waits rewrites any
instruction Tile scheduled with 2+.
"""

import numpy as np
from contextlib import ExitStack

import concourse.bass as bass
import concourse.mybir as mybir
from concourse.tile import TileContext
from concourse.masks import make_identity
from concourse.bass_utils import run_bass_kernel_spmd

F32 = mybir.dt.float32
F32R = mybir.dt.float32r
BF16 = mybir.dt.bfloat16
AF = mybir.ActivationFunctionType
OP = mybir.AluOpType

N, FIN, H, FOUT = 4096, 128, 4, 64
G = H * FOUT
NCORES = 8
NLOC = N // NCORES          # local target rows per core
NCH = N // 128              # source (m) chunks
LCH = NLOC // 128           # local output row chunks
HE = FOUT + 1               # hu_ext columns (u column at index FOUT)
XPC = 8                     # phase-1 chunks per xT DMA piece
NXP = NCH // XPC            # xT DMA pieces

# GpSimd tensor ops contend catastrophically with concurrent DVE ops
# (measured: DVE TS slows 12x while a gp TT runs), so ALL elementwise
# work stays on the DVE.
GP_BLK = []
DV_BLK = [(0, 4), (4, 4), (8, 8), (16, 8), (24, 8)]  # DVE TT slabs
ALL_BLK = DV_BLK
START_C = 0
STOP_C = NCH - 1


def build_program():
    # cpack: xT | xTloc | wsrc | wproj|wtgt | wskip   (bf16)
    cw = N + NLOC + H + G + H + G

    nc = bass.Bass()
    d_cpack = nc.declare_dram_parameter("cpack", [128, cw], BF16, isOutput=False)
    d_mask = nc.declare_dram_parameter("mask01", [N, NLOC], BF16, isOutput=False)
    d_sel = nc.declare_dram_parameter("selc", [4, 4 * 128], BF16, isOutput=False)
    d_bias = nc.declare_dram_parameter("biasr", [1, G], F32R, isOutput=False)
    d_out = nc.declare_dram_parameter("out", [NLOC, G], F32, isOutput=True)

    with TileContext(nc) as tc, ExitStack() as ctx:
        cp = ctx.enter_context(tc.tile_pool(name="const", bufs=1))
        sb_cpack = cp.tile([128, cw], BF16, tag="cpack")
        o = 0
        xTr = sb_cpack[:, o:o + N]; o += N
        xTlocr = sb_cpack[:, o:o + NLOC]; o += NLOC
        wsrcr = sb_cpack[:, o:o + H]; o += H
        wpsr = sb_cpack[:, o:o + G + H]; o += G + H      # wproj | wtgt merged
        wskipr = sb_cpack[:, o:o + G]; o += G
        browr = cp.tile([1, G], F32R, tag="brow")

        sb_sel = cp.tile([4, 4 * 128], BF16, tag="sel")   # one-hot head rows
        sb_id = cp.tile([128, 128], F32, tag="ident")
        sb_hp = cp.tile([128, NCH * H * HE], BF16, tag="hext")   # h | 1
        sb_w = cp.tile([128, H * NLOC], BF16, tag="wbc")      # e^{0.8 s_src}
        sb_wrow = cp.tile([4, NLOC], BF16, tag="wrow")
        sb_ones1 = cp.tile([1, 128], F32R, tag="ones1")
        sb_q = cp.tile([128, NCH * H], F32, tag="qexp")       # e^{-0.8 s_tgt}
        sb_u = cp.tile([128, NCH * H], F32, tag="uexp")       # e^{s_tgt}
        sb_m = [cp.tile([128, nb * NLOC], BF16, tag=f"m{k}", name=f"m{k}")
                for k, (j0, nb) in enumerate(ALL_BLK)]

        # h_ext view [128, c, h, HE]
        hp4 = sb_hp[:].rearrange("p (c h w) -> p c h w", h=H, w=HE)

        # ---- DMA: everything big rides the SP HWDGE queue, xT pieces
        # interleaved with mask slabs (each xT piece lands well before the
        # phase-1 chunk that needs it; the Act queue is NOT used for bulk
        # data because its DMA_DIRECT2D dispatch occupies the Act engine
        # for ~2.5us per slab, starving phase-1 exps/copies).
        def dma_mask(k):
            j0, nb = ALL_BLK[k]
            mv = sb_m[k][:].rearrange("p (c n) -> p c n", n=NLOC)
            dv = d_mask[j0 * 128:(j0 + nb) * 128, :].rearrange(
                "(c p) n -> p c n", p=128)
            nc.sync.dma_start(out=mv, in_=dv)

        nc.sync.dma_start(out=sb_cpack[:, N:N + NLOC + H],
                          in_=d_cpack[:, N:N + NLOC + H])
        nc.sync.dma_start(out=sb_cpack[:, N + NLOC + H:cw],
                          in_=d_cpack[:, N + NLOC + H:cw])
        nc.scalar.dma_start(out=sb_sel[:], in_=d_sel[:])
        nc.scalar.dma_start(out=browr[:], in_=d_bias[:])
        mj = 0
        for p in range(NXP):
            w0 = p * XPC * 128
            nc.sync.dma_start(out=sb_cpack[:, w0:w0 + XPC * 128],
                              in_=d_cpack[:, w0:w0 + XPC * 128])
            while mj < len(ALL_BLK) and mj <= 2 * p:
                dma_mask(mj); mj += 1
        while mj < len(ALL_BLK):
            dma_mask(mj); mj += 1

        make_identity(nc, sb_id[:])
        nc.vector.memset(sb_ones1[:].bitcast(F32), 1.0)
        # ones column of h_ext: the scaled hu copy then yields u*1 = u in
        # the denominator column for free
        nc.vector.memset(hp4[:, :, :, FOUT:FOUT + 1], 1.0)

        def q_ap(c, hh):
            return sb_q[:, c * H + hh:c * H + hh + 1]

        def hu_lhsT(c, hh):
            return hp4[:, c:c + 1, hh:hh + 1, 0:HE]

        # ---- phase 0: b = s_src(local), w = e^{0.8 b} broadcast -----------
        # (pso opened first so po banks coexist with phase-1 ph banks)
        pso = ctx.enter_context(tc.tile_pool(name="pso", bufs=1, space="PSUM"))
        po = [pso.tile([HE, NLOC], F32, tag=f"po{hh}", name=f"po{hh}")
              for hh in range(H)]

        with tc.tile_pool(name="ps0", bufs=1, space="PSUM") as ps0:
            pb = ps0.tile([4, NLOC], F32, tag="pb")
            nc.tensor.matmul(pb[:], wsrcr, xTlocr, start=True, stop=True)
            nc.scalar.activation(sb_wrow[:], pb[:], AF.Exp, scale=0.8)
            for hh in range(H):
                pwb = ps0.tile([128, NLOC], F32, tag=f"pwb{hh % 2}",
                               name=f"pwb{hh}")
                nc.tensor.matmul(pwb[:], sb_sel[0:4, hh * 128:(hh + 1) * 128],
                                 sb_wrow[0:4, :], start=True, stop=True)
                nc.scalar.copy(sb_w[:, hh * NLOC:(hh + 1) * NLOC], pwb[:])

        # et slabs for the attention loop: written per-chunk by the DVE
        # scalar_tensor_tensor, consumed per-chunk by the PE.  bufs=2 so the
        # next head's DVE work overlaps this head's PE consumption.
        wpt = ctx.enter_context(tc.tile_pool(name="workt2", bufs=1))
        wp = ctx.enter_context(tc.tile_pool(name="work", bufs=2))
        fp = ctx.enter_context(tc.tile_pool(name="fin", bufs=1))
        fp2 = ctx.enter_context(tc.tile_pool(name="fin2", bufs=2))
        afs = [fp.tile([128, G], F32, tag=f"af{li}", name=f"af{li}")
               for li in range(LCH)]

        def blk_of(c):
            return next(k for k, (j0, nb) in enumerate(ALL_BLK)
                        if j0 <= c < j0 + nb)

        t2s = {}             # (hh, k) -> t2 slab (all blocks)
        ets = {}             # (hh, k) -> et slab

        def emit_ts(hh, c):
            # t2 = max(w, q) on DVE (2x mode), written into the block slab
            k = blk_of(c)
            j0, nb = ALL_BLK[k]
            if (hh, k) not in t2s:
                t2s[(hh, k)] = wpt.tile([128, nb * NLOC], BF16,
                                        tag=f"t2b{k}", name=f"t2_{hh}_{k}")
            t2 = t2s[(hh, k)]
            nc.vector.tensor_scalar(t2[:, (c - j0) * NLOC:(c - j0 + 1) * NLOC],
                                    sb_w[:, hh * NLOC:(hh + 1) * NLOC],
                                    q_ap(c, hh),
                                    sb_u[:, c * H + hh:c * H + hh + 1],
                                    OP.max, OP.mult)

        def emit_tt(hh, k):
            # et = t2 * M01 for a whole block: GpSimd for the GP blocks,
            # DVE for the rest
            j0, nb = ALL_BLK[k]
            et = wp.tile([128, nb * NLOC], BF16, tag=f"etb{k}",
                         name=f"et_{hh}_{k}")
            nc.vector.tensor_tensor(et[:], t2s[(hh, k)][:], sb_m[k][:], OP.mult)
            del t2s[(hh, k)]
            ets[(hh, k)] = et

        def emit_agg_blk(hh, k, start_c=START_C, stop_c=STOP_C):
            j0, nb = ALL_BLK[k]
            et = ets.pop((hh, k))
            for s in range(nb):
                c = j0 + s
                nc.tensor.matmul(po[hh][:], hu_lhsT(c, hh),
                                 et[:, s * NLOC:(s + 1) * NLOC],
                                 start=(c == start_c), stop=(c == stop_c))

        # ---- phase 1 fused with head 0 ------------------------------------
        # per chunk j: proj matmul -> u/q exps + h copy (Scalar) -> hu(head0)
        # + head-0 t2 (DVE); block TT fires at each block end (gpsimd TTs run
        # far ahead of their end-of-chain consumption).  DVE-block aggs trail
        # one chunk; gp-block aggs all run after the loop.
        with tc.tile_pool(name="ps1", bufs=4, space="PSUM") as ps1:
            for j in range(NCH):
                ph = ps1.tile([128, G + H], F32, tag="ph")
                nc.tensor.matmul(ph[:], xTr[:, j * 128:(j + 1) * 128], wpsr,
                                 start=True, stop=True)
                nc.scalar.activation(sb_u[:, j * H:(j + 1) * H],
                                     ph[:, G:G + H], AF.Exp)
                nc.scalar.activation(sb_q[:, j * H:(j + 1) * H],
                                     ph[:, G:G + H], AF.Exp, scale=-0.8)
                nc.scalar.copy(
                    hp4[:, j, :, 0:FOUT],
                    ph[:, 0:G].rearrange("p (h f) -> p h f", f=FOUT))
                emit_ts(0, j)
                for k, (j0, nb) in enumerate(ALL_BLK):
                    if j == j0 + nb - 1:
                        emit_tt(0, k)
                    if j == j0 + nb + 1:      # block k's aggs, trailing
                        emit_agg_blk(0, k)
            emit_agg_blk(0, len(ALL_BLK) - 1)

        # ---- heads 1..3 + per-head finalize -------------------------------
        pos_all = []
        with tc.tile_pool(name="psf", bufs=2, space="PSUM") as psf, \
             tc.tile_pool(name="psk", bufs=1, space="PSUM") as psk:
            # skip+bias for all li, hoisted off the tail critical path (the
            # PE runs these during head 1; results wait in PSUM)
            pskipb = psk.tile([128, LCH * G], F32, tag="pskipb")
            for li in range(LCH):
                nc.tensor.matmul(pskipb[:, li * G:(li + 1) * G],
                                 xTlocr[:, li * 128:(li + 1) * 128],
                                 wskipr, start=True, stop=False,
                                 skip_group_check=True)
                nc.tensor.matmul(pskipb[:, li * G:(li + 1) * G],
                                 sb_ones1[:], browr[0:1, :],
                                 start=False, stop=True, skip_group_check=True)

            def head_finalize(hh):
                # copy this head's accumulator out of PSUM so the PE can
                # transpose from SBUF, then per-li: transpose, 1/den, scaled
                # copy into af columns.  All overlapped with the next head.
                pos = cp.tile([HE, NLOC], F32, tag=f"pos{hh}", name=f"pos{hh}")
                for li in range(LCH):
                    nc.scalar.copy(pos[:, li * 128:(li + 1) * 128],
                                   po[hh][:, li * 128:(li + 1) * 128])
                pos_all.append(pos)
                ptb = psf.tile([128, LCH * HE], F32, tag="pt")
                for li in range(LCH):
                    nc.tensor.transpose(ptb[0:128, li * HE:li * HE + HE],
                                        pos[:, li * 128:(li + 1) * 128],
                                        sb_id[0:HE, 0:HE])
                rcpb = fp2.tile([128, LCH], F32, tag="rcp")
                nc.vector.reciprocal(
                    rcpb[:].rearrange("p (l o) -> p l o", o=1),
                    ptb[:].rearrange("p (l w) -> p l w", w=HE)[:, :, FOUT:FOUT + 1])
                for li in range(LCH):
                    nc.scalar.activation(afs[li][:, hh * FOUT:(hh + 1) * FOUT],
                                         ptb[:, li * HE:li * HE + FOUT],
                                         AF.Copy, scale=rcpb[:, li:li + 1])

            for hh in range(1, H):
                # DVE blocks: t2 + TT + aggs.  Head 3 rotates to end on the
                # small (4,4) block so the tail after its last DVE op is short.
                order = [2, 3, 4, 0, 1] if hh == H - 1 else list(range(len(ALL_BLK)))
                st = ALL_BLK[order[0]][0]
                sp_j0, sp_nb = ALL_BLK[order[-1]]
                sp = sp_j0 + sp_nb - 1
                for k in order:
                    j0, nb = ALL_BLK[k]
                    for s in range(nb):
                        emit_ts(hh, j0 + s)
                    emit_tt(hh, k)
                    emit_agg_blk(hh, k, start_c=st, stop_c=sp)
                # previous head's finalize AFTER this head's PE chain: its
                # transposes sit behind these matmuls in the PE FIFO, so by
                # the time the PE reaches them the pos copy has long landed.
                head_finalize(hh - 1)
            head_finalize(H - 1)

            # ---- tail: ELU + store (per li, pipelined) --------------------
            for li in range(LCH):
                af = afs[li]
                nc.vector.tensor_tensor(af[:], af[:],
                                        pskipb[:, li * G:(li + 1) * G], OP.add)
                # ELU(z) = relu(z) + exp(-relu(-z)) - 1, relu/exp on Scalar
                rp = fp2.tile([128, G], F32, tag="rp")
                nc.scalar.activation(rp[:], af[:], AF.Relu)
                rn = fp2.tile([128, G], F32, tag="rn")
                nc.scalar.activation(rn[:], af[:], AF.Relu, scale=-1.0)
                ex = fp2.tile([128, G], F32, tag="ex")
                nc.scalar.activation(ex[:], rn[:], AF.Exp, scale=-1.0)
                nc.vector.tensor_tensor(af[:], rp[:], ex[:], OP.add)
                nc.vector.tensor_scalar(af[:], af[:], -1.0, None, OP.add)
                nc.sync.dma_start(out=d_out[li * 128:(li + 1) * 128, :], in_=af[:])

    _split_multi_waits(nc)
    return nc


def _split_multi_waits(nc):
    """walrus on this toolchain allows only one semaphore-wait command on
    most compute-engine instructions (S3_LW / S3D3_* structs).  Tile's
    scheduler freely emits 2+.  Move all but one wait onto an injected
    same-engine NoOp right before the offending instruction."""
    skip = (mybir.InstEventSemaphore,)
    k = 0
    for f in nc.m.functions:
        for blk in f.blocks:
            new = []
            for ins in blk.instructions:
                si = getattr(ins, "sync_info", None)
                w = list(si.on_wait) if si is not None and si.on_wait else []
                if len(w) > 1 and not isinstance(ins, skip):
                    for wx in w[:-1]:
                        nop = mybir.InstNoOp(name=f"waitsplit-{k}", ins=[], outs=[])
                        nop.engine = ins.engine
                        nop.sync_info = mybir.SyncInfo(on_wait=[wx], on_update=[])
                        new.append(nop)
                        k += 1
                    ins.sync_info = mybir.SyncInfo(on_wait=w[-1:],
                                                   on_update=list(si.on_update))
                new.append(ins)
            blk.instructions[:] = new


_PROG = None


def _get_prog():
    global _PROG
    if _PROG is None:
        _PROG = build_program()
    return _PROG


def make_in_maps(x, mask, proj_param, score_src, score_tgt, skip_w, bias):
    import ml_dtypes
    x = np.asarray(x, np.float32)
    mask = np.asarray(mask, np.float32)
    proj = np.asarray(proj_param, np.float32)
    a_src = np.asarray(score_src, np.float32)[:, :, 0]       # [H, FOUT]
    a_tgt = np.asarray(score_tgt, np.float32)[:, :, 0]
    skip = np.asarray(skip_w, np.float32)
    b = np.asarray(bias, np.float32)

    xT = np.ascontiguousarray(x.T)                           # [128, N]
    wproj = np.ascontiguousarray(proj.transpose(1, 0, 2).reshape(FIN, G))
    w_src = np.einsum('hif,hf->ih', proj, a_src)             # [FIN, H]
    w_tgt = np.einsum('hif,hf->ih', proj, a_tgt)
    wskip = np.ascontiguousarray(skip.T)                     # [128, G]
    mask01 = (mask == 0.0).astype(ml_dtypes.bfloat16)        # [N, N]

    sel = np.zeros((4, 4 * 128), ml_dtypes.bfloat16)
    for hh in range(H):
        sel[hh, hh * 128:(hh + 1) * 128] = 1

    in_maps = []
    for c in range(NCORES):
        r0 = c * NLOC
        cpack = np.concatenate(
            [xT, xT[:, r0:r0 + NLOC], w_src, wproj, w_tgt, wskip],
            axis=1).astype(ml_dtypes.bfloat16)
        in_maps.append({
            "cpack": np.ascontiguousarray(cpack),
            "mask01": np.ascontiguousarray(mask01[r0:r0 + NLOC, :].T),
            "selc": sel,
            "biasr": b.reshape(1, G).astype(np.float32),
        })
    return in_maps


def run(in_maps, trace=False, **kw):
    res = run_bass_kernel_spmd(_get_prog(), in_maps, list(range(NCORES)),
                               trace=trace, **kw)
    out = np.concatenate([res.results[c]["out"] for c in range(NCORES)], axis=0)
    return out, res


def kernel(x, mask, proj_param, score_src, score_tgt, skip_w, bias):
    in_maps = make_in_maps(x, mask, proj_param, score_src, score_tgt, skip_w, bias)
    out, _ = run(in_maps)
    return out.astype(np.float32)


# revision 41
# speedup vs baseline: 1.0279x; 1.0069x over previous
"""GAT (graph attention) forward on 8 TRN2 NeuronCores, Bass/Tile.

Sharding: target nodes (rows of the output) split into 8 blocks of 512.
Each core redundantly computes the projected features h for ALL nodes
(cheap: one K=128 matmul chain) and then its own 512-row slice of the
attention + aggregation + skip + ELU.  No collectives.

Score factorization trick: with z[m,n] = s_tgt[m] + s_src[n] and
leaky(z) = max(z, 0.2 z),

    exp(leaky(z)) = e^{0.2 s_src[n]} * u_m * max(w_n, q_m)

where u = e^{s_tgt}, q = e^{-0.8 s_tgt}, w = e^{0.8 s_src}.  The leading
per-target factor cancels in the softmax normalization, so per (head,
chunk) the kernel runs one 2-op DVE tensor_scalar
(t2 = (w max q) * u, 2x perf mode) and one mask tensor_tensor
(et = t2 * M01, 2x mode, slab-batched).  This is the DVE wall: cayman
has no 2x uop for the fused scalar_tensor_tensor (measured 1x), GpSimd
tensor ops contend ~12x with concurrent DVE work, and the Act engine
cannot multiply two tensors - so ~1 masked-scored element per
lane-cycle is the hard elementwise floor and the DVE paces the kernel.

The aggregation matmul streams et against the stationary h_ext (h with
a ones column) so the softmax denominator falls out of the same PSUM
accumulation; a per-head transpose + batched reciprocal + scaled copy
assembles the normalized output columns.

Head 0's elementwise + aggregation is fused into the phase-1
projection loop (only 3 head periods are exposed), the skip+bias
matmuls are hoisted off the tail into PSUM during head 1, and the ELU
runs as relu/exp legs on the Act engine with only two DVE ops.

Everything matmul runs in bf16 (scores included; rel-err ~3.4e-3
against the 2e-2 budget), which halves the input DMA and enables FWL
weight loads.  The mask is transposed and converted to multiplicative
0/1 bf16 on the host so the device only ever does contiguous row DMA.
All bulk DMA rides the SP HWDGE queue - mask slabs interleaved with xT
pieces so each lands just before its first consumer; the Act queue is
NOT used for bulk data because its DMA_DIRECT2D dispatch occupies the
Act engine ~2.5us per slab.  The S3_LW (weight-load) instruction can
carry only one semaphore wait, so _split_multi_waits rewrites any
instruction Tile scheduled with 2+.
"""

import numpy as np
from contextlib import ExitStack

import concourse.bass as bass
import concourse.mybir as mybir
from concourse.tile import TileContext
from concourse.masks import make_identity
from concourse.bass_utils import run_bass_kernel_spmd

F32 = mybir.dt.float32
F32R = mybir.dt.float32r
BF16 = mybir.dt.bfloat16
AF = mybir.ActivationFunctionType
OP = mybir.AluOpType

N, FIN, H, FOUT = 4096, 128, 4, 64
G = H * FOUT
NCORES = 8
NLOC = N // NCORES          # local target rows per core
NCH = N // 128              # source (m) chunks
LCH = NLOC // 128           # local output row chunks
HE = FOUT + 1               # hu_ext columns (u column at index FOUT)
XPC = 8                     # phase-1 chunks per xT DMA piece
NXP = NCH // XPC            # xT DMA pieces

# GpSimd tensor ops contend catastrophically with concurrent DVE ops
# (measured: DVE TS slows 12x while a gp TT runs), so ALL elementwise
# work stays on the DVE.
GP_BLK = []
DV_BLK = [(0, 4), (4, 4), (8, 8), (16, 8), (24, 8)]  # DVE TT slabs
ALL_BLK = DV_BLK
START_C = 0
STOP_C = NCH - 1


def build_program():
    # cpack: xT | xTloc | wsrc | wproj|wtgt | wskip   (bf16)
    cw = N + NLOC + H + G + H + G

    nc = bass.Bass()
    d_cpack = nc.declare_dram_parameter("cpack", [128, cw], BF16, isOutput=False)
    d_mask = nc.declare_dram_parameter("mask01", [N, NLOC], BF16, isOutput=False)
    d_sel = nc.declare_dram_parameter("selc", [4, 4 * 128], BF16, isOutput=False)
    d_bias = nc.declare_dram_parameter("biasr", [1, G], F32R, isOutput=False)
    d_out = nc.declare_dram_parameter("out", [NLOC, G], BF16, isOutput=True)

    with TileContext(nc) as tc, ExitStack() as ctx:
        cp = ctx.enter_context(tc.tile_pool(name="const", bufs=1))
        sb_cpack = cp.tile([128, cw], BF16, tag="cpack")
        o = 0
        xTr = sb_cpack[:, o:o + N]; o += N
        xTlocr = sb_cpack[:, o:o + NLOC]; o += NLOC
        wsrcr = sb_cpack[:, o:o + H]; o += H
        wpsr = sb_cpack[:, o:o + G + H]; o += G + H      # wproj | wtgt merged
        wskipr = sb_cpack[:, o:o + G]; o += G
        browr = cp.tile([1, G], F32R, tag="brow")

        sb_sel = cp.tile([4, 4 * 128], BF16, tag="sel")   # one-hot head rows
        sb_id = cp.tile([128, 128], F32, tag="ident")
        sb_hp = cp.tile([128, NCH * H * HE], BF16, tag="hext")   # h | 1
        sb_w = cp.tile([128, H * NLOC], BF16, tag="wbc")      # e^{0.8 s_src}
        sb_wrow = cp.tile([4, NLOC], BF16, tag="wrow")
        sb_ones1 = cp.tile([1, 128], F32R, tag="ones1")
        sb_q = cp.tile([128, NCH * H], F32, tag="qexp")       # e^{-0.8 s_tgt}
        sb_u = cp.tile([128, NCH * H], F32, tag="uexp")       # e^{s_tgt}
        sb_m = [cp.tile([128, nb * NLOC], BF16, tag=f"m{k}", name=f"m{k}")
                for k, (j0, nb) in enumerate(ALL_BLK)]

        # h_ext view [128, c, h, HE]
        hp4 = sb_hp[:].rearrange("p (c h w) -> p c h w", h=H, w=HE)

        # ---- DMA: everything big rides the SP HWDGE queue, xT pieces
        # interleaved with mask slabs (each xT piece lands well before the
        # phase-1 chunk that needs it; the Act queue is NOT used for bulk
        # data because its DMA_DIRECT2D dispatch occupies the Act engine
        # for ~2.5us per slab, starving phase-1 exps/copies).
        def dma_mask(k):
            j0, nb = ALL_BLK[k]
            mv = sb_m[k][:].rearrange("p (c n) -> p c n", n=NLOC)
            dv = d_mask[j0 * 128:(j0 + nb) * 128, :].rearrange(
                "(c p) n -> p c n", p=128)
            nc.sync.dma_start(out=mv, in_=dv)

        nc.sync.dma_start(out=sb_cpack[:, N:N + NLOC + H],
                          in_=d_cpack[:, N:N + NLOC + H])
        nc.sync.dma_start(out=sb_cpack[:, N + NLOC + H:cw],
                          in_=d_cpack[:, N + NLOC + H:cw])
        nc.scalar.dma_start(out=sb_sel[:], in_=d_sel[:])
        nc.scalar.dma_start(out=browr[:], in_=d_bias[:])
        mj = 0
        for p in range(NXP):
            w0 = p * XPC * 128
            nc.sync.dma_start(out=sb_cpack[:, w0:w0 + XPC * 128],
                              in_=d_cpack[:, w0:w0 + XPC * 128])
            while mj < len(ALL_BLK) and mj <= 2 * p:
                dma_mask(mj); mj += 1
        while mj < len(ALL_BLK):
            dma_mask(mj); mj += 1

        make_identity(nc, sb_id[:])
        nc.vector.memset(sb_ones1[:].bitcast(F32), 1.0)
        # ones column of h_ext: the scaled hu copy then yields u*1 = u in
        # the denominator column for free
        nc.vector.memset(hp4[:, :, :, FOUT:FOUT + 1], 1.0)

        def q_ap(c, hh):
            return sb_q[:, c * H + hh:c * H + hh + 1]

        def hu_lhsT(c, hh):
            return hp4[:, c:c + 1, hh:hh + 1, 0:HE]

        # ---- phase 0: b = s_src(local), w = e^{0.8 b} broadcast -----------
        # (pso opened first so po banks coexist with phase-1 ph banks)
        pso = ctx.enter_context(tc.tile_pool(name="pso", bufs=1, space="PSUM"))
        po = [pso.tile([HE, NLOC], F32, tag=f"po{hh}", name=f"po{hh}")
              for hh in range(H)]

        with tc.tile_pool(name="ps0", bufs=1, space="PSUM") as ps0:
            pb = ps0.tile([4, NLOC], F32, tag="pb")
            nc.tensor.matmul(pb[:], wsrcr, xTlocr, start=True, stop=True)
            nc.scalar.activation(sb_wrow[:], pb[:], AF.Exp, scale=0.8)
            for hh in range(H):
                pwb = ps0.tile([128, NLOC], F32, tag=f"pwb{hh % 2}",
                               name=f"pwb{hh}")
                nc.tensor.matmul(pwb[:], sb_sel[0:4, hh * 128:(hh + 1) * 128],
                                 sb_wrow[0:4, :], start=True, stop=True)
                nc.scalar.copy(sb_w[:, hh * NLOC:(hh + 1) * NLOC], pwb[:])

        # et slabs for the attention loop: written per-chunk by the DVE
        # scalar_tensor_tensor, consumed per-chunk by the PE.  bufs=2 so the
        # next head's DVE work overlaps this head's PE consumption.
        wpt = ctx.enter_context(tc.tile_pool(name="workt2", bufs=1))
        wp = ctx.enter_context(tc.tile_pool(name="work", bufs=2))
        fp = ctx.enter_context(tc.tile_pool(name="fin", bufs=1))
        fp2 = ctx.enter_context(tc.tile_pool(name="fin2", bufs=2))
        afs = [fp.tile([128, G], F32, tag=f"af{li}", name=f"af{li}")
               for li in range(LCH)]

        def blk_of(c):
            return next(k for k, (j0, nb) in enumerate(ALL_BLK)
                        if j0 <= c < j0 + nb)

        t2s = {}             # (hh, k) -> t2 slab (all blocks)
        ets = {}             # (hh, k) -> et slab

        def emit_ts(hh, c):
            # t2 = max(w, q) on DVE (2x mode), written into the block slab
            k = blk_of(c)
            j0, nb = ALL_BLK[k]
            if (hh, k) not in t2s:
                t2s[(hh, k)] = wpt.tile([128, nb * NLOC], BF16,
                                        tag=f"t2b{k}", name=f"t2_{hh}_{k}")
            t2 = t2s[(hh, k)]
            nc.vector.tensor_scalar(t2[:, (c - j0) * NLOC:(c - j0 + 1) * NLOC],
                                    sb_w[:, hh * NLOC:(hh + 1) * NLOC],
                                    q_ap(c, hh),
                                    sb_u[:, c * H + hh:c * H + hh + 1],
                                    OP.max, OP.mult)

        def emit_tt(hh, k):
            # et = t2 * M01 for a whole block: GpSimd for the GP blocks,
            # DVE for the rest
            j0, nb = ALL_BLK[k]
            et = wp.tile([128, nb * NLOC], BF16, tag=f"etb{k}",
                         name=f"et_{hh}_{k}")
            nc.vector.tensor_tensor(et[:], t2s[(hh, k)][:], sb_m[k][:], OP.mult)
            del t2s[(hh, k)]
            ets[(hh, k)] = et

        def emit_agg_blk(hh, k, start_c=START_C, stop_c=STOP_C):
            j0, nb = ALL_BLK[k]
            et = ets.pop((hh, k))
            for s in range(nb):
                c = j0 + s
                nc.tensor.matmul(po[hh][:], hu_lhsT(c, hh),
                                 et[:, s * NLOC:(s + 1) * NLOC],
                                 start=(c == start_c), stop=(c == stop_c))

        # ---- phase 1 fused with head 0 ------------------------------------
        # per chunk j: proj matmul -> u/q exps + h copy (Scalar) -> hu(head0)
        # + head-0 t2 (DVE); block TT fires at each block end (gpsimd TTs run
        # far ahead of their end-of-chain consumption).  DVE-block aggs trail
        # one chunk; gp-block aggs all run after the loop.
        with tc.tile_pool(name="ps1", bufs=4, space="PSUM") as ps1:
            for j in range(NCH):
                ph = ps1.tile([128, G + H], F32, tag="ph")
                nc.tensor.matmul(ph[:], xTr[:, j * 128:(j + 1) * 128], wpsr,
                                 start=True, stop=True)
                nc.scalar.activation(sb_u[:, j * H:(j + 1) * H],
                                     ph[:, G:G + H], AF.Exp)
                nc.scalar.activation(sb_q[:, j * H:(j + 1) * H],
                                     ph[:, G:G + H], AF.Exp, scale=-0.8)
                nc.scalar.copy(
                    hp4[:, j, :, 0:FOUT],
                    ph[:, 0:G].rearrange("p (h f) -> p h f", f=FOUT))
                emit_ts(0, j)
                for k, (j0, nb) in enumerate(ALL_BLK):
                    if j == j0 + nb - 1:
                        emit_tt(0, k)
                    if j == j0 + nb + 1:      # block k's aggs, trailing
                        emit_agg_blk(0, k)
            emit_agg_blk(0, len(ALL_BLK) - 1)

        # ---- heads 1..3 + per-head finalize -------------------------------
        pos_all = []
        with tc.tile_pool(name="psf", bufs=2, space="PSUM") as psf, \
             tc.tile_pool(name="psk", bufs=1, space="PSUM") as psk:
            # skip+bias for all li, hoisted off the tail critical path (the
            # PE runs these during head 1; results wait in PSUM)
            pskipb = psk.tile([128, LCH * G], F32, tag="pskipb")
            for li in range(LCH):
                nc.tensor.matmul(pskipb[:, li * G:(li + 1) * G],
                                 xTlocr[:, li * 128:(li + 1) * 128],
                                 wskipr, start=True, stop=False,
                                 skip_group_check=True)
                nc.tensor.matmul(pskipb[:, li * G:(li + 1) * G],
                                 sb_ones1[:], browr[0:1, :],
                                 start=False, stop=True, skip_group_check=True)

            def head_finalize(hh):
                # copy this head's accumulator out of PSUM so the PE can
                # transpose from SBUF, then per-li: transpose, 1/den, scaled
                # copy into af columns.  All overlapped with the next head.
                pos = cp.tile([HE, NLOC], F32, tag=f"pos{hh}", name=f"pos{hh}")
                for li in range(LCH):
                    nc.scalar.copy(pos[:, li * 128:(li + 1) * 128],
                                   po[hh][:, li * 128:(li + 1) * 128])
                pos_all.append(pos)
                ptb = psf.tile([128, LCH * HE], F32, tag="pt")
                for li in range(LCH):
                    nc.tensor.transpose(ptb[0:128, li * HE:li * HE + HE],
                                        pos[:, li * 128:(li + 1) * 128],
                                        sb_id[0:HE, 0:HE])
                rcpb = fp2.tile([128, LCH], F32, tag="rcp")
                nc.vector.reciprocal(
                    rcpb[:].rearrange("p (l o) -> p l o", o=1),
                    ptb[:].rearrange("p (l w) -> p l w", w=HE)[:, :, FOUT:FOUT + 1])
                for li in range(LCH):
                    nc.scalar.activation(afs[li][:, hh * FOUT:(hh + 1) * FOUT],
                                         ptb[:, li * HE:li * HE + FOUT],
                                         AF.Copy, scale=rcpb[:, li:li + 1])

            for hh in range(1, H):
                # DVE blocks: t2 + TT + aggs.  Head 3 rotates to end on the
                # small (4,4) block so the tail after its last DVE op is short.
                order = [2, 3, 4, 0, 1] if hh == H - 1 else list(range(len(ALL_BLK)))
                st = ALL_BLK[order[0]][0]
                sp_j0, sp_nb = ALL_BLK[order[-1]]
                sp = sp_j0 + sp_nb - 1
                for k in order:
                    j0, nb = ALL_BLK[k]
                    for s in range(nb):
                        emit_ts(hh, j0 + s)
                    emit_tt(hh, k)
                    emit_agg_blk(hh, k, start_c=st, stop_c=sp)
                # previous head's finalize AFTER this head's PE chain: its
                # transposes sit behind these matmuls in the PE FIFO, so by
                # the time the PE reaches them the pos copy has long landed.
                head_finalize(hh - 1)
            head_finalize(H - 1)

            # ---- tail: ELU + store (per li, pipelined) --------------------
            for li in range(LCH):
                af = afs[li]
                nc.vector.tensor_tensor(af[:], af[:],
                                        pskipb[:, li * G:(li + 1) * G], OP.add)
                # ELU(z) = relu(z) + exp(-relu(-z)) - 1, relu/exp on Scalar
                rp = fp2.tile([128, G], F32, tag="rp")
                nc.scalar.activation(rp[:], af[:], AF.Relu)
                rn = fp2.tile([128, G], F32, tag="rn")
                nc.scalar.activation(rn[:], af[:], AF.Relu, scale=-1.0)
                ex = fp2.tile([128, G], F32, tag="ex")
                nc.scalar.activation(ex[:], rn[:], AF.Exp, scale=-1.0)
                nc.vector.tensor_tensor(af[:], rp[:], ex[:], OP.add)
                afb = fp2.tile([128, G], BF16, tag="afb")
                nc.vector.tensor_scalar(afb[:], af[:], -1.0, None, OP.add)
                nc.sync.dma_start(out=d_out[li * 128:(li + 1) * 128, :], in_=afb[:])

    _split_multi_waits(nc)
    return nc


def _split_multi_waits(nc):
    """walrus on this toolchain allows only one semaphore-wait command on
    most compute-engine instructions (S3_LW / S3D3_* structs).  Tile's
    scheduler freely emits 2+.  Move all but one wait onto an injected
    same-engine NoOp right before the offending instruction."""
    skip = (mybir.InstEventSemaphore,)
    k = 0
    for f in nc.m.functions:
        for blk in f.blocks:
            new = []
            for ins in blk.instructions:
                si = getattr(ins, "sync_info", None)
                w = list(si.on_wait) if si is not None and si.on_wait else []
                if len(w) > 1 and not isinstance(ins, skip):
                    for wx in w[:-1]:
                        nop = mybir.InstNoOp(name=f"waitsplit-{k}", ins=[], outs=[])
                        nop.engine = ins.engine
                        nop.sync_info = mybir.SyncInfo(on_wait=[wx], on_update=[])
                        new.append(nop)
                        k += 1
                    ins.sync_info = mybir.SyncInfo(on_wait=w[-1:],
                                                   on_update=list(si.on_update))
                new.append(ins)
            blk.instructions[:] = new


_PROG = None


def _get_prog():
    global _PROG
    if _PROG is None:
        _PROG = build_program()
    return _PROG


def make_in_maps(x, mask, proj_param, score_src, score_tgt, skip_w, bias):
    import ml_dtypes
    x = np.asarray(x, np.float32)
    mask = np.asarray(mask, np.float32)
    proj = np.asarray(proj_param, np.float32)
    a_src = np.asarray(score_src, np.float32)[:, :, 0]       # [H, FOUT]
    a_tgt = np.asarray(score_tgt, np.float32)[:, :, 0]
    skip = np.asarray(skip_w, np.float32)
    b = np.asarray(bias, np.float32)

    xT = np.ascontiguousarray(x.T)                           # [128, N]
    wproj = np.ascontiguousarray(proj.transpose(1, 0, 2).reshape(FIN, G))
    w_src = np.einsum('hif,hf->ih', proj, a_src)             # [FIN, H]
    w_tgt = np.einsum('hif,hf->ih', proj, a_tgt)
    wskip = np.ascontiguousarray(skip.T)                     # [128, G]
    mask01 = (mask == 0.0).astype(ml_dtypes.bfloat16)        # [N, N]

    sel = np.zeros((4, 4 * 128), ml_dtypes.bfloat16)
    for hh in range(H):
        sel[hh, hh * 128:(hh + 1) * 128] = 1

    in_maps = []
    for c in range(NCORES):
        r0 = c * NLOC
        cpack = np.concatenate(
            [xT, xT[:, r0:r0 + NLOC], w_src, wproj, w_tgt, wskip],
            axis=1).astype(ml_dtypes.bfloat16)
        in_maps.append({
            "cpack": np.ascontiguousarray(cpack),
            "mask01": np.ascontiguousarray(mask01[r0:r0 + NLOC, :].T),
            "selc": sel,
            "biasr": b.reshape(1, G).astype(np.float32),
        })
    return in_maps


def run(in_maps, trace=False, **kw):
    res = run_bass_kernel_spmd(_get_prog(), in_maps, list(range(NCORES)),
                               trace=trace, **kw)
    out = np.concatenate([res.results[c]["out"] for c in range(NCORES)], axis=0)
    return out, res


def kernel(x, mask, proj_param, score_src, score_tgt, skip_w, bias):
    in_maps = make_in_maps(x, mask, proj_param, score_src, score_tgt, skip_w, bias)
    out, _ = run(in_maps)
    return out.astype(np.float32)


# revision 44
# speedup vs baseline: 1.0384x; 1.0102x over previous
"""GAT (graph attention) forward on 8 TRN2 NeuronCores, Bass/Tile.

Sharding: target nodes (rows of the output) split into 8 blocks of 512.
Each core redundantly computes the projected features h for ALL nodes
(cheap: one K=128 matmul chain) and then its own 512-row slice of the
attention + aggregation + skip + ELU.  No collectives.

Score factorization trick: with z[m,n] = s_tgt[m] + s_src[n] and
leaky(z) = max(z, 0.2 z),

    exp(leaky(z)) = e^{0.2 s_src[n]} * u_m * max(w_n, q_m)

where u = e^{s_tgt}, q = e^{-0.8 s_tgt}, w = e^{0.8 s_src}.  The leading
per-target factor cancels in the softmax normalization, so per (head,
chunk) the kernel runs one 2-op DVE tensor_scalar
(t2 = (w max q) * u, 2x perf mode) and one mask tensor_tensor
(et = t2 * M01, 2x mode, slab-batched).  This is the DVE wall: cayman
has no 2x uop for the fused scalar_tensor_tensor (measured 1x), GpSimd
tensor ops contend ~12x with concurrent DVE work, and the Act engine
cannot multiply two tensors - so ~1 masked-scored element per
lane-cycle is the hard elementwise floor and the DVE paces the kernel.

The aggregation matmul streams et against the stationary h_ext (h with
a ones column) so the softmax denominator falls out of the same PSUM
accumulation; a per-head transpose + batched reciprocal + scaled copy
assembles the normalized output columns.

Head 0's elementwise + aggregation is fused into the phase-1
projection loop (only 3 head periods are exposed), the skip+bias
matmuls are hoisted off the tail into PSUM during head 1, and the ELU
runs as relu/exp legs on the Act engine with only two DVE ops.

Everything matmul runs in bf16 (scores included; rel-err ~3.4e-3
against the 2e-2 budget), which halves the input DMA and enables FWL
weight loads.  The mask is transposed and converted to multiplicative
0/1 bf16 on the host so the device only ever does contiguous row DMA.
All bulk DMA rides the SP HWDGE queue - mask slabs interleaved with xT
pieces so each lands just before its first consumer; the Act queue is
NOT used for bulk data because its DMA_DIRECT2D dispatch occupies the
Act engine ~2.5us per slab.  The S3_LW (weight-load) instruction can
carry only one semaphore wait, so _split_multi_waits rewrites any
instruction Tile scheduled with 2+.
"""

import numpy as np
from contextlib import ExitStack

import concourse.bass as bass
import concourse.mybir as mybir
from concourse.tile import TileContext
from concourse.masks import make_identity
from concourse.bass_utils import run_bass_kernel_spmd

F32 = mybir.dt.float32
F32R = mybir.dt.float32r
BF16 = mybir.dt.bfloat16
AF = mybir.ActivationFunctionType
OP = mybir.AluOpType

N, FIN, H, FOUT = 4096, 128, 4, 64
G = H * FOUT
NCORES = 8
NLOC = N // NCORES          # local target rows per core
NCH = N // 128              # source (m) chunks
LCH = NLOC // 128           # local output row chunks
HE = FOUT + 1               # hu_ext columns (u column at index FOUT)
XPC = 8                     # phase-1 chunks per xT DMA piece
NXP = NCH // XPC            # xT DMA pieces

# GpSimd tensor ops contend catastrophically with concurrent DVE ops
# (measured: DVE TS slows 12x while a gp TT runs), so ALL elementwise
# work stays on the DVE.
GP_BLK = []
DV_BLK = [(0, 4), (4, 4), (8, 8), (16, 8), (24, 8)]  # DVE TT slabs
ALL_BLK = DV_BLK
START_C = 0
STOP_C = NCH - 1


def build_program():
    # cpack: xT | xTloc | wsrc | wproj|wtgt | wskip   (bf16)
    cw = N + NLOC + H + G + H + G

    nc = bass.Bass()
    d_cpack = nc.declare_dram_parameter("cpack", [128, cw], BF16, isOutput=False)
    d_mask = nc.declare_dram_parameter("mask01", [N, NLOC], BF16, isOutput=False)
    d_sel = nc.declare_dram_parameter("selc", [4, 4 * 128], BF16, isOutput=False)
    d_bias = nc.declare_dram_parameter("biasr", [1, G], F32R, isOutput=False)
    d_out = nc.declare_dram_parameter("out", [NLOC, G], BF16, isOutput=True)

    with TileContext(nc) as tc, ExitStack() as ctx:
        cp = ctx.enter_context(tc.tile_pool(name="const", bufs=1))
        sb_cpack = cp.tile([128, cw], BF16, tag="cpack")
        o = 0
        xTr = sb_cpack[:, o:o + N]; o += N
        xTlocr = sb_cpack[:, o:o + NLOC]; o += NLOC
        wsrcr = sb_cpack[:, o:o + H]; o += H
        wpsr = sb_cpack[:, o:o + G + H]; o += G + H      # wproj | wtgt merged
        wskipr = sb_cpack[:, o:o + G]; o += G
        browr = cp.tile([1, G], F32R, tag="brow")

        sb_sel = cp.tile([4, 4 * 128], BF16, tag="sel")   # one-hot head rows
        sb_id = cp.tile([128, 128], F32, tag="ident")
        sb_hp = cp.tile([128, NCH * H * HE], BF16, tag="hext")   # h | 1
        sb_w = cp.tile([128, H * NLOC], BF16, tag="wbc")      # e^{0.8 s_src}
        sb_wrow = cp.tile([4, NLOC], BF16, tag="wrow")
        sb_ones1 = cp.tile([1, 128], F32R, tag="ones1")
        sb_q = cp.tile([128, NCH * H], F32, tag="qexp")       # e^{-0.8 s_tgt}
        sb_u = cp.tile([128, NCH * H], F32, tag="uexp")       # e^{s_tgt}
        sb_m = [cp.tile([128, nb * NLOC], BF16, tag=f"m{k}", name=f"m{k}")
                for k, (j0, nb) in enumerate(ALL_BLK)]

        # h_ext view [128, c, h, HE]
        hp4 = sb_hp[:].rearrange("p (c h w) -> p c h w", h=H, w=HE)

        # ---- DMA: everything big rides the SP HWDGE queue, xT pieces
        # interleaved with mask slabs (each xT piece lands well before the
        # phase-1 chunk that needs it; the Act queue is NOT used for bulk
        # data because its DMA_DIRECT2D dispatch occupies the Act engine
        # for ~2.5us per slab, starving phase-1 exps/copies).
        def dma_mask(k):
            j0, nb = ALL_BLK[k]
            mv = sb_m[k][:].rearrange("p (c n) -> p c n", n=NLOC)
            dv = d_mask[j0 * 128:(j0 + nb) * 128, :].rearrange(
                "(c p) n -> p c n", p=128)
            nc.sync.dma_start(out=mv, in_=dv)

        nc.sync.dma_start(out=sb_cpack[:, N:N + NLOC + H],
                          in_=d_cpack[:, N:N + NLOC + H])
        nc.sync.dma_start(out=sb_cpack[:, N + NLOC + H:cw],
                          in_=d_cpack[:, N + NLOC + H:cw])
        nc.scalar.dma_start(out=sb_sel[:], in_=d_sel[:])
        nc.scalar.dma_start(out=browr[:], in_=d_bias[:])
        mj = 0
        for p in range(NXP):
            w0 = p * XPC * 128
            nc.sync.dma_start(out=sb_cpack[:, w0:w0 + XPC * 128],
                              in_=d_cpack[:, w0:w0 + XPC * 128])
            while mj < len(ALL_BLK) and mj <= 2 * p:
                dma_mask(mj); mj += 1
        while mj < len(ALL_BLK):
            dma_mask(mj); mj += 1

        make_identity(nc, sb_id[:])
        nc.vector.memset(sb_ones1[:].bitcast(F32), 1.0)
        # ones column of h_ext: the scaled hu copy then yields u*1 = u in
        # the denominator column for free
        nc.vector.memset(hp4[:, :, :, FOUT:FOUT + 1], 1.0)

        def q_ap(c, hh):
            return sb_q[:, c * H + hh:c * H + hh + 1]

        def hu_lhsT(c, hh):
            return hp4[:, c:c + 1, hh:hh + 1, 0:HE]

        # ---- phase 0: b = s_src(local), w = e^{0.8 b} broadcast -----------
        # (pso opened first so po banks coexist with phase-1 ph banks)
        pso = ctx.enter_context(tc.tile_pool(name="pso", bufs=1, space="PSUM"))
        po = [pso.tile([HE, NLOC], F32, tag=f"po{hh}", name=f"po{hh}")
              for hh in range(H)]

        with tc.tile_pool(name="ps0", bufs=1, space="PSUM") as ps0:
            pb = ps0.tile([4, NLOC], F32, tag="pb")
            nc.tensor.matmul(pb[:], wsrcr, xTlocr, start=True, stop=True)
            nc.scalar.activation(sb_wrow[:], pb[:], AF.Exp, scale=0.8)
            for hh in range(H):
                pwb = ps0.tile([128, NLOC], F32, tag=f"pwb{hh % 2}",
                               name=f"pwb{hh}")
                nc.tensor.matmul(pwb[:], sb_sel[0:4, hh * 128:(hh + 1) * 128],
                                 sb_wrow[0:4, :], start=True, stop=True)
                nc.scalar.copy(sb_w[:, hh * NLOC:(hh + 1) * NLOC], pwb[:])

        # et slabs for the attention loop: written per-chunk by the DVE
        # scalar_tensor_tensor, consumed per-chunk by the PE.  bufs=2 so the
        # next head's DVE work overlaps this head's PE consumption.
        wpt = ctx.enter_context(tc.tile_pool(name="workt2", bufs=1))
        wp = ctx.enter_context(tc.tile_pool(name="work", bufs=2))
        fp = ctx.enter_context(tc.tile_pool(name="fin", bufs=1))
        fp2 = ctx.enter_context(tc.tile_pool(name="fin2", bufs=2))
        afs = [fp.tile([128, G], F32, tag=f"af{li}", name=f"af{li}")
               for li in range(LCH)]

        def blk_of(c):
            return next(k for k, (j0, nb) in enumerate(ALL_BLK)
                        if j0 <= c < j0 + nb)

        t2s = {}             # (hh, k) -> t2 slab (all blocks)
        ets = {}             # (hh, k) -> et slab

        def emit_ts(hh, c):
            # t2 = max(w, q) on DVE (2x mode), written into the block slab
            k = blk_of(c)
            j0, nb = ALL_BLK[k]
            if (hh, k) not in t2s:
                t2s[(hh, k)] = wpt.tile([128, nb * NLOC], BF16,
                                        tag=f"t2b{k}", name=f"t2_{hh}_{k}")
            t2 = t2s[(hh, k)]
            nc.vector.tensor_scalar(t2[:, (c - j0) * NLOC:(c - j0 + 1) * NLOC],
                                    sb_w[:, hh * NLOC:(hh + 1) * NLOC],
                                    q_ap(c, hh),
                                    sb_u[:, c * H + hh:c * H + hh + 1],
                                    OP.max, OP.mult)

        def emit_tt(hh, k):
            # et = t2 * M01 for a whole block: GpSimd for the GP blocks,
            # DVE for the rest
            j0, nb = ALL_BLK[k]
            et = wp.tile([128, nb * NLOC], BF16, tag=f"etb{k}",
                         name=f"et_{hh}_{k}")
            nc.vector.tensor_tensor(et[:], t2s[(hh, k)][:], sb_m[k][:], OP.mult)
            del t2s[(hh, k)]
            ets[(hh, k)] = et

        def emit_agg_blk(hh, k, start_c=START_C, stop_c=STOP_C):
            j0, nb = ALL_BLK[k]
            et = ets.pop((hh, k))
            for s in range(nb):
                c = j0 + s
                nc.tensor.matmul(po[hh][:], hu_lhsT(c, hh),
                                 et[:, s * NLOC:(s + 1) * NLOC],
                                 start=(c == start_c), stop=(c == stop_c))

        # ---- phase 1 fused with head 0 ------------------------------------
        # per chunk j: proj matmul -> u/q exps + h copy (Scalar) -> hu(head0)
        # + head-0 t2 (DVE); block TT fires at each block end (gpsimd TTs run
        # far ahead of their end-of-chain consumption).  DVE-block aggs trail
        # one chunk; gp-block aggs all run after the loop.
        with tc.tile_pool(name="ps1", bufs=4, space="PSUM") as ps1:
            for j in range(NCH):
                ph = ps1.tile([128, G + H], F32, tag="ph")
                nc.tensor.matmul(ph[:], xTr[:, j * 128:(j + 1) * 128], wpsr,
                                 start=True, stop=True)
                nc.scalar.activation(sb_u[:, j * H:(j + 1) * H],
                                     ph[:, G:G + H], AF.Exp)
                nc.scalar.activation(sb_q[:, j * H:(j + 1) * H],
                                     ph[:, G:G + H], AF.Exp, scale=-0.8)
                nc.scalar.copy(
                    hp4[:, j, :, 0:FOUT],
                    ph[:, 0:G].rearrange("p (h f) -> p h f", f=FOUT))
                emit_ts(0, j)
                for k, (j0, nb) in enumerate(ALL_BLK):
                    if j == j0 + nb - 1:
                        emit_tt(0, k)
                    if j == j0 + nb + 1:      # block k's aggs, trailing
                        emit_agg_blk(0, k)
            emit_agg_blk(0, len(ALL_BLK) - 1)

        # ---- heads 1..3 + per-head finalize -------------------------------
        pos_all = []
        with tc.tile_pool(name="psf", bufs=2, space="PSUM") as psf, \
             tc.tile_pool(name="psk", bufs=1, space="PSUM") as psk:
            # skip+bias for all li, hoisted off the tail critical path (the
            # PE runs these during head 1; results wait in PSUM)
            pskipb = psk.tile([128, LCH * G], F32, tag="pskipb")
            for li in range(LCH):
                nc.tensor.matmul(pskipb[:, li * G:(li + 1) * G],
                                 xTlocr[:, li * 128:(li + 1) * 128],
                                 wskipr, start=True, stop=False,
                                 skip_group_check=True)
                nc.tensor.matmul(pskipb[:, li * G:(li + 1) * G],
                                 sb_ones1[:], browr[0:1, :],
                                 start=False, stop=True, skip_group_check=True)

            def head_finalize(hh, last=False):
                # copy this head's accumulator out of PSUM so the PE can
                # transpose from SBUF, then per-li: transpose, 1/den, scaled
                # copy into af columns.  All overlapped with the next head.
                pos = cp.tile([HE, NLOC], F32, tag=f"pos{hh}", name=f"pos{hh}")
                for li in range(LCH):
                    nc.scalar.copy(pos[:, li * 128:(li + 1) * 128],
                                   po[hh][:, li * 128:(li + 1) * 128])
                pos_all.append(pos)
                ptb = psf.tile([128, LCH * HE], F32, tag="pt")
                if last:
                    # per-li pipelined: each li's transpose/recip/scaled-copy
                    # starts as soon as its pos slice lands (tail latency)
                    for li in range(LCH):
                        nc.tensor.transpose(ptb[0:128, li * HE:li * HE + HE],
                                            pos[:, li * 128:(li + 1) * 128],
                                            sb_id[0:HE, 0:HE])
                        rcp1 = fp2.tile([128, 1], F32, tag="rcp1",
                                        name=f"rcp1_{li}")
                        nc.vector.reciprocal(
                            rcp1[:], ptb[:, li * HE + FOUT:li * HE + FOUT + 1])
                        nc.scalar.activation(
                            afs[li][:, hh * FOUT:(hh + 1) * FOUT],
                            ptb[:, li * HE:li * HE + FOUT],
                            AF.Copy, scale=rcp1[:])
                    return
                for li in range(LCH):
                    nc.tensor.transpose(ptb[0:128, li * HE:li * HE + HE],
                                        pos[:, li * 128:(li + 1) * 128],
                                        sb_id[0:HE, 0:HE])
                rcpb = fp2.tile([128, LCH], F32, tag="rcp")
                nc.vector.reciprocal(
                    rcpb[:].rearrange("p (l o) -> p l o", o=1),
                    ptb[:].rearrange("p (l w) -> p l w", w=HE)[:, :, FOUT:FOUT + 1])
                for li in range(LCH):
                    nc.scalar.activation(afs[li][:, hh * FOUT:(hh + 1) * FOUT],
                                         ptb[:, li * HE:li * HE + FOUT],
                                         AF.Copy, scale=rcpb[:, li:li + 1])

            for hh in range(1, H):
                # DVE blocks: t2 + TT + aggs.  Head 3 rotates to end on the
                # small (4,4) block so the tail after its last DVE op is short.
                order = [2, 3, 4, 0, 1] if hh == H - 1 else list(range(len(ALL_BLK)))
                st = ALL_BLK[order[0]][0]
                sp_j0, sp_nb = ALL_BLK[order[-1]]
                sp = sp_j0 + sp_nb - 1
                for k in order:
                    j0, nb = ALL_BLK[k]
                    for s in range(nb):
                        emit_ts(hh, j0 + s)
                    emit_tt(hh, k)
                    emit_agg_blk(hh, k, start_c=st, stop_c=sp)
                # previous head's finalize AFTER this head's PE chain: its
                # transposes sit behind these matmuls in the PE FIFO, so by
                # the time the PE reaches them the pos copy has long landed.
                head_finalize(hh - 1)
            head_finalize(H - 1, last=True)

            # ---- tail: ELU + store (per li, pipelined) --------------------
            for li in range(LCH):
                af = afs[li]
                nc.vector.tensor_tensor(af[:], af[:],
                                        pskipb[:, li * G:(li + 1) * G], OP.add)
                # ELU(z) = relu(z) + exp(-relu(-z)) - 1, relu/exp on Scalar
                rp = fp2.tile([128, G], F32, tag="rp")
                nc.scalar.activation(rp[:], af[:], AF.Relu)
                mn = fp2.tile([128, G], F32, tag="mn")
                nc.vector.tensor_scalar(mn[:], af[:], 0.0, None, OP.min)
                ex = fp2.tile([128, G], F32, tag="ex")
                nc.scalar.activation(ex[:], mn[:], AF.Exp)
                nc.vector.tensor_tensor(af[:], rp[:], ex[:], OP.add)
                afb = fp2.tile([128, G], BF16, tag="afb")
                nc.vector.tensor_scalar(afb[:], af[:], -1.0, None, OP.add)
                nc.sync.dma_start(out=d_out[li * 128:(li + 1) * 128, :], in_=afb[:])

    _split_multi_waits(nc)
    return nc


def _split_multi_waits(nc):
    """walrus on this toolchain allows only one semaphore-wait command on
    most compute-engine instructions (S3_LW / S3D3_* structs).  Tile's
    scheduler freely emits 2+.  Move all but one wait onto an injected
    same-engine NoOp right before the offending instruction."""
    skip = (mybir.InstEventSemaphore,)
    k = 0
    for f in nc.m.functions:
        for blk in f.blocks:
            new = []
            for ins in blk.instructions:
                si = getattr(ins, "sync_info", None)
                w = list(si.on_wait) if si is not None and si.on_wait else []
                if len(w) > 1 and not isinstance(ins, skip):
                    for wx in w[:-1]:
                        nop = mybir.InstNoOp(name=f"waitsplit-{k}", ins=[], outs=[])
                        nop.engine = ins.engine
                        nop.sync_info = mybir.SyncInfo(on_wait=[wx], on_update=[])
                        new.append(nop)
                        k += 1
                    ins.sync_info = mybir.SyncInfo(on_wait=w[-1:],
                                                   on_update=list(si.on_update))
                new.append(ins)
            blk.instructions[:] = new


_PROG = None


def _get_prog():
    global _PROG
    if _PROG is None:
        _PROG = build_program()
    return _PROG


def make_in_maps(x, mask, proj_param, score_src, score_tgt, skip_w, bias):
    import ml_dtypes
    x = np.asarray(x, np.float32)
    mask = np.asarray(mask, np.float32)
    proj = np.asarray(proj_param, np.float32)
    a_src = np.asarray(score_src, np.float32)[:, :, 0]       # [H, FOUT]
    a_tgt = np.asarray(score_tgt, np.float32)[:, :, 0]
    skip = np.asarray(skip_w, np.float32)
    b = np.asarray(bias, np.float32)

    xT = np.ascontiguousarray(x.T)                           # [128, N]
    wproj = np.ascontiguousarray(proj.transpose(1, 0, 2).reshape(FIN, G))
    w_src = np.einsum('hif,hf->ih', proj, a_src)             # [FIN, H]
    w_tgt = np.einsum('hif,hf->ih', proj, a_tgt)
    wskip = np.ascontiguousarray(skip.T)                     # [128, G]
    mask01 = (mask == 0.0).astype(ml_dtypes.bfloat16)        # [N, N]

    sel = np.zeros((4, 4 * 128), ml_dtypes.bfloat16)
    for hh in range(H):
        sel[hh, hh * 128:(hh + 1) * 128] = 1

    in_maps = []
    for c in range(NCORES):
        r0 = c * NLOC
        cpack = np.concatenate(
            [xT, xT[:, r0:r0 + NLOC], w_src, wproj, w_tgt, wskip],
            axis=1).astype(ml_dtypes.bfloat16)
        in_maps.append({
            "cpack": np.ascontiguousarray(cpack),
            "mask01": np.ascontiguousarray(mask01[r0:r0 + NLOC, :].T),
            "selc": sel,
            "biasr": b.reshape(1, G).astype(np.float32),
        })
    return in_maps


def run(in_maps, trace=False, **kw):
    res = run_bass_kernel_spmd(_get_prog(), in_maps, list(range(NCORES)),
                               trace=trace, **kw)
    out = np.concatenate([res.results[c]["out"] for c in range(NCORES)], axis=0)
    return out, res


def kernel(x, mask, proj_param, score_src, score_tgt, skip_w, bias):
    in_maps = make_in_maps(x, mask, proj_param, score_src, score_tgt, skip_w, bias)
    out, _ = run(in_maps)
    return out.astype(np.float32)
